# revision 1
# baseline (speedup 1.0000x reference)
"""DotGAT layer (segment-softmax GNN message passing) on 8 Trainium2 cores, v2.

Strategy (graph/data parallel per the sharding hint):
  - Nodes split into 8 contiguous ranges of 6272 (49 blocks of 128); each core
    owns the edges whose dst falls in its range.  Halo exchange is host-side
    data layout: each core receives its edges' source features as two streams,
    feature-major fp16 (logit path) and edge-major fp8 (value path), grouped
    by dst block and padded to 128-edge chunks.
  - Logits use the fused form e = z_src . (B z_dst) with B = Wk Wq^T, so q/k
    are never materialised: per block one matmul produces qk = B^T? applied to
    own-node features; per chunk one matmul contracts the feature-major source
    stream against it.
  - The segment softmax is masked pre-exp: a device-generated one-hot
    (iota vs dst column, scaled by 30/tau) is accumulated into the logit PSUM
    via an identity matmul, then one ACT exp per chunk-pair produces
    P[e,n] = exp(tau*S - 30 + 30*[n==dst_e]); off-dst entries are ~1e-13 and
    vanish in the aggregation.  The uniform exp(~) factor on dst entries
    cancels in the softmax normalisation.
  - Aggregation accumulates RAW source features: U[g,n] += zet_chunk^T P and
    den[n] += P^T 1 per chunk; per block one matmul projects num = U^T Wv and
    a reciprocal-scale gives h.  (Biases are zero for this problem; asserted.)

The program is recompiled per call with all data-dependent sizes baked in
(SPMD: one instruction stream, 8 cores).
"""

import sys

sys.path.insert(0, "/opt/trn_rl_repo")

import numpy as np
import ml_dtypes

N_NODES = 50000
N_EDGES_EXPECT = 800000
DIM = 128
N_CORES = 8
BLK = 128
BLOCKS_PER_CORE = 49
NODES_PER_CORE = BLOCKS_PER_CORE * BLK  # 6272
N_PAD = NODES_PER_CORE * N_CORES  # 50176
TAU = 1.0 / np.sqrt(DIM)
MASKC = 30.0 / TAU  # one-hot scale so exp sees +30 on the dst entry

F8 = ml_dtypes.float8_e4m3


def _prepare(z, Wq, bq, Wk, bk, Wv, bv, src, dst):
    """Host-side sharding: per-core edge grouping and the two source-feature
    streams (pure data movement / layout, no arithmetic)."""
    z = np.asarray(z, np.float32)
    src = np.asarray(src, np.int32)
    dst = np.asarray(dst, np.int32)
    assert not np.any(np.asarray(bq)) and not np.any(np.asarray(bk)) and not np.any(
        np.asarray(bv)
    ), "v2 kernel assumes zero biases"

    # feature-major z (fp16) with one zero column for edge padding, and
    # row-major fp8 copy for the value path
    zT16 = np.zeros((DIM, N_PAD + 1), np.float16)
    zT16[:, :N_NODES] = z.T.astype(np.float16)
    z16 = np.zeros((N_PAD + 1, DIM), np.float16)
    z16[:N_NODES] = z.astype(np.float16)

    per_core = []
    for c in range(N_CORES):
        n0 = c * NODES_PER_CORE
        sel = (dst >= n0) & (dst < n0 + NODES_PER_CORE)
        es = src[sel].astype(np.int64)
        ed = (dst[sel] - n0).astype(np.int64)
        blk = ed >> 7
        order = np.lexsort((ed, blk))
        es, ed, blk = es[order], ed[order], blk[order]
        cnt = np.zeros(BLOCKS_PER_CORE, np.int64)
        np.add.at(cnt, blk, 1)
        per_core.append(dict(es=es, ed=ed, cnt=cnt))

    cnts = np.stack([pc["cnt"] for pc in per_core])  # [8, 49]
    C = (-(-cnts // BLK)).max(axis=0)  # [49] per-position chunk counts
    S = int(C.sum())
    offs = np.concatenate([[0], np.cumsum(C)]).astype(int)

    WqT = np.ascontiguousarray(np.asarray(Wq, np.float32).T).astype(np.float16)
    WkT = np.ascontiguousarray(np.asarray(Wk, np.float32).T).astype(np.float16)
    Wv16 = np.asarray(Wv, np.float32).astype(np.float16)
    I8 = np.eye(DIM, dtype=np.float32).astype(F8)
    iota_row = np.broadcast_to(
        np.arange(BLK, dtype=np.float16), (BLK, BLK)
    ).copy()  # [e, n] value = n

    in_maps = []
    for c in range(N_CORES):
        pc = per_core[c]
        es, ed, cnt = pc["es"], pc["ed"], pc["cnt"]
        col = np.full(S * BLK, N_PAD, np.int64)  # pad -> zero feature row/col
        drel = np.full(S * BLK, -1.0, np.float32)  # pad -> matches no node
        ptr = 0
        for b in range(BLOCKS_PER_CORE):
            n = int(cnt[b])
            base = int(offs[b]) * BLK
            col[base : base + n] = es[ptr : ptr + n]
            drel[base : base + n] = (ed[ptr : ptr + n] - b * BLK).astype(np.float32)
            ptr += n
        ze = np.ascontiguousarray(zT16[:, col])  # [128, S*128] fp16
        zet = np.ascontiguousarray(
            z16[col].reshape(S, BLK, DIM).transpose(1, 0, 2).reshape(BLK, S * DIM)
        )  # [128(e), S*128(g)] fp16
        dstT = np.ascontiguousarray(
            drel.reshape(S, BLK).T.astype(np.float32)
        )  # [128(e), S]
        zq = np.ascontiguousarray(zT16[:, c * NODES_PER_CORE : (c + 1) * NODES_PER_CORE])
        in_maps.append(
            dict(
                ze=ze,
                zet=zet,
                dstT=dstT,
                zq=zq,
                WqT=WqT,
                WkT=WkT,
                Wv=Wv16,
                I8=I8,
                iota=iota_row,
            )
        )
    consts = dict(C=[int(x) for x in C], S=S)
    return in_maps, consts


def _build(consts):
    import concourse.bacc as bacc
    import concourse.mybir as mybir
    import concourse.tile as tile

    dt = mybir.dt
    Alu = mybir.AluOpType
    Act = mybir.ActivationFunctionType

    C = consts["C"]
    S = consts["S"]
    Cmax = max(C)
    offs = np.concatenate([[0], np.cumsum(C)]).astype(int)

    nc = bacc.Bacc("TRN2", target_bir_lowering=False, debug=False, num_devices=N_CORES)

    ze = nc.declare_dram_parameter("ze", [128, S * BLK], dt.float16, isOutput=False)
    zet = nc.declare_dram_parameter("zet", [128, S * BLK], dt.float16, isOutput=False)
    dstT = nc.declare_dram_parameter("dstT", [128, S], dt.float32, isOutput=False)
    zq = nc.declare_dram_parameter("zq", [128, NODES_PER_CORE], dt.float16, isOutput=False)
    WqT = nc.declare_dram_parameter("WqT", [128, 128], dt.float16, isOutput=False)
    WkT = nc.declare_dram_parameter("WkT", [128, 128], dt.float16, isOutput=False)
    Wv = nc.declare_dram_parameter("Wv", [128, 128], dt.float16, isOutput=False)
    I8 = nc.declare_dram_parameter("I8", [128, 128], dt.float8e4, isOutput=False)
    iota = nc.declare_dram_parameter("iota", [128, 128], dt.float16, isOutput=False)
    h = nc.declare_dram_parameter("h", [NODES_PER_CORE, DIM], dt.float16, isOutput=True)

    with tile.TileContext(nc) as tc:
        with tc.tile_pool(name="const", bufs=1) as constp:
            wqt_sb = constp.tile([128, 128], dt.float16)
            wkt_sb = constp.tile([128, 128], dt.float16)
            wv_sb = constp.tile([128, 128], dt.float16)
            i8_sb = constp.tile([128, 128], dt.float8e4)
            iota_sb = constp.tile([128, 128], dt.float16)
            dst_sb = constp.tile([128, S], dt.float32)
            ones_sb = constp.tile([128, 1], dt.float16)
            negb_sb = constp.tile([128, 1], dt.float32)
            nc.vector.memset(negb_sb[:], -30.0)
            nc.sync.dma_start(wqt_sb[:], WqT[:])
            nc.sync.dma_start(wkt_sb[:], WkT[:])
            nc.sync.dma_start(wv_sb[:], Wv[:])
            nc.sync.dma_start(i8_sb[:], I8[:])
            nc.sync.dma_start(iota_sb[:], iota[:])
            nc.sync.dma_start(dst_sb[:], dstT[:])
            nc.vector.memset(ones_sb[:], 1.0)

            # X[j, i] = B^T[j, i] = sum_g Wq[j,g] Wk[i,g]; per block then
            # qk[i, n] = sum_j X[j, i] z_own[j, n]
            x_sb = constp.tile([128, 128], dt.float16)

            # ---- PE warm-up: ~6us of dense matmuls so the HAM clock gate
            # lifts the PE to 2.4 GHz before the main loop ----
            with tc.tile_pool(name="warm", bufs=4, space="PSUM") as wpool:
                for i in range(80):
                    wps = wpool.tile([128, 128], dt.float32, tag="w")
                    nc.tensor.matmul(
                        wps[:], lhsT=wqt_sb[:], rhs=wkt_sb[:], start=True, stop=True
                    )
                xp = wpool.tile([128, 128], dt.float32, tag="w")
                nc.tensor.matmul(xp[:], lhsT=wqt_sb[:], rhs=wkt_sb[:], start=True, stop=True)
                nc.scalar.copy(x_sb[:], xp[:])

            G = 4  # blocks per DMA group (amortise per-dma fixed cost)
            zq_all = constp.tile([128, NODES_PER_CORE], dt.float16)
            nc.sync.dma_start(zq_all[:], zq[:])
            with (
                tc.tile_pool(name="zep", bufs=2) as zep,
                tc.tile_pool(name="zetp", bufs=2) as zetp,
                tc.tile_pool(name="qkp", bufs=2) as qkp,
                tc.tile_pool(name="mp", bufs=6) as mp,
                tc.tile_pool(name="ptp", bufs=3) as ptp,
                tc.tile_pool(name="usb", bufs=2) as usbp,
                tc.tile_pool(name="recp", bufs=2) as recp,
                tc.tile_pool(name="hp", bufs=2) as hp,
                tc.tile_pool(name="stps", bufs=2, space="PSUM") as stps,
                tc.tile_pool(name="bps", bufs=2, space="PSUM") as bps,
                tc.tile_pool(name="ups", bufs=2, space="PSUM") as ups,
            ):
                # ---- per-block state built as we stream ----
                pending = []  # (block_idx, cc, pt_tile, col_off) awaiting MM3/4
                blk_state = {}

                def open_block(b, zs, zt):
                    Cc = C[b]
                    bk = bps.tile([128, 512], dt.float32, tag="blk")
                    u_ps = ups.tile([128, 128], dt.float32, tag="u")
                    nc.tensor.matmul(
                        bk[:, 384:512], lhsT=x_sb[:],
                        rhs=zq_all[:, b * 128 : (b + 1) * 128], start=True, stop=True
                    )
                    qk_sb = qkp.tile([128, 128], dt.float16, tag="qksb")
                    nc.scalar.copy(qk_sb[:], bk[:, 384:512])
                    blk_state[b] = dict(
                        zs=zs, zt=zt, qk=qk_sb[:], u=u_ps[:], d=bk[:, 128:129],
                        num=bk[:, 256:384], emitted=0, Cc=Cc
                    )

                def emit_mm34(b, cc, pt_half):
                    st = blk_state[b]
                    first = st["emitted"] == 0
                    last = st["emitted"] == st["Cc"] - 1
                    nc.tensor.matmul(
                        st["u"],
                        lhsT=st["zt"][:, cc * 128 : (cc + 1) * 128],
                        rhs=pt_half,
                        start=first,
                        stop=last,
                    )
                    nc.tensor.matmul(
                        st["d"],
                        lhsT=pt_half,
                        rhs=ones_sb[:],
                        start=first,
                        stop=last,
                    )
                    st["emitted"] += 1
                    if last:
                        close_block(b)

                def close_block(b):
                    st = blk_state.pop(b)
                    u_sb = usbp.tile([128, 128], dt.float16, tag="usb")
                    nc.vector.tensor_copy(u_sb[:], st["u"])
                    nc.tensor.matmul(
                        st["num"], lhsT=u_sb[:], rhs=wv_sb[:], start=True, stop=True
                    )
                    rec = recp.tile([128, 1], dt.float32, tag="rec")
                    nc.vector.reciprocal(rec[:], st["d"])
                    ht = hp.tile([128, 128], dt.float16, tag="h")
                    nc.vector.tensor_scalar(
                        out=ht[:], in0=st["num"], scalar1=rec[:], scalar2=None,
                        op0=Alu.mult,
                    )
                    nc.sync.dma_start(h[b * 128 : (b + 1) * 128, :], ht[:])

                groups = [
                    list(range(g, min(g + G, BLOCKS_PER_CORE)))
                    for g in range(0, BLOCKS_PER_CORE, G)
                ]
                for grp in groups:
                    off0 = int(offs[grp[0]])
                    totC = sum(C[b] for b in grp)
                    zeg = zep.tile([128, G * Cmax * 128], dt.float16, tag="ze")
                    nc.sync.dma_start(
                        zeg[:, : totC * 128], ze[:, off0 * 128 : (off0 + totC) * 128]
                    )
                    ztg = zetp.tile([128, G * Cmax * 128], dt.float16, tag="zet")
                    nc.sync.dma_start(
                        ztg[:, : totC * 128], zet[:, off0 * 128 : (off0 + totC) * 128]
                    )
                    loc = 0
                    for b in grp:
                        Cc = C[b]
                        if Cc == 0:
                            ht = hp.tile([128, 128], dt.float16, tag="h")
                            nc.vector.memset(ht[:], 0.0)
                            nc.sync.dma_start(h[b * 128 : (b + 1) * 128, :], ht[:])
                            continue
                        zs = zeg[:, loc * 128 : (loc + Cc) * 128]
                        zt = ztg[:, loc * 128 : (loc + Cc) * 128]
                        loc += Cc
                        open_block(b, zs, zt)
                        st_blk = blk_state[b]
                        off = int(offs[b])
                        for c0 in range(0, Cc, 4):
                            nq = min(4, Cc - c0)
                            w = nq * 128
                            stt = stps.tile([128, 512], dt.float32, tag="st")
                            ptt = ptp.tile([128, 512], dt.float16, tag="pt")
                            for j in range(nq):
                                cc = c0 + j
                                m = mp.tile([128, 128], dt.float16, tag="m")
                                nc.vector.tensor_scalar(
                                    out=m[:], in0=iota_sb[:],
                                    scalar1=dst_sb[:, off + cc : off + cc + 1],
                                    scalar2=MASKC, op0=Alu.is_equal, op1=Alu.mult,
                                )
                                nc.tensor.matmul(
                                    stt[:, j * 128 : (j + 1) * 128],
                                    lhsT=st_blk["zs"][:, cc * 128 : (cc + 1) * 128],
                                    rhs=st_blk["qk"],
                                    start=True, stop=False,
                                )
                                nc.tensor.matmul(
                                    stt[:, j * 128 : (j + 1) * 128],
                                    lhsT=i8_sb[:], rhs=m[:],
                                    start=False, stop=True,
                                )
                            nc.scalar.activation(
                                ptt[:, 0:w], stt[:, 0:w], Act.Exp, bias=negb_sb[:],
                                scale=float(TAU),
                            )
                            for j in range(nq):
                                pending.append(
                                    (b, c0 + j, ptt[:, j * 128 : (j + 1) * 128])
                                )
                            while len(pending) > 4:
                                pb, pcc, ph = pending.pop(0)
                                emit_mm34(pb, pcc, ph)
                while pending:
                    pb, pcc, ph = pending.pop(0)
                    emit_mm34(pb, pcc, ph)

    nc.compile()
    return nc


def _install_ntff_hook():
    """The agent image's antenv lacks axon_hooks; recreate it and register
    the ctypes NTFF profile hook the boot would have installed."""
    import types

    if "antenv.axon_hooks" not in sys.modules:
        import antenv

        m = types.ModuleType("antenv.axon_hooks")
        m._hook = None
        m.set_axon_ntff_profile_hook = lambda h, _m=m: setattr(_m, "_hook", h)
        m.get_axon_ntff_profile_hook = lambda _m=m: _m._hook
        sys.modules["antenv.axon_hooks"] = m
        antenv.axon_hooks = m
    from antenv import axon_hooks

    if axon_hooks.get_axon_ntff_profile_hook() is None:
        from trn_agent_boot.trn_boot import _ntff_profile_via_ctypes

        hook = _ntff_profile_via_ctypes("/opt/axon/libaxon_pjrt.so")
        if hook is not None:
            axon_hooks.set_axon_ntff_profile_hook(hook)


def run(inputs, trace=False):
    """Returns (h [50000,128] float32, exec_time_ns or None)."""
    from concourse.bass_utils import run_bass_kernel_spmd

    if trace:
        try:
            _install_ntff_hook()
        except Exception as e:  # profiling is best-effort
            print(f"ntff hook install failed: {e}", file=sys.stderr)

    in_maps, consts = _prepare(**inputs)
    nc = _build(consts)
    res = run_bass_kernel_spmd(
        nc,
        [dict(m) for m in in_maps],
        list(range(N_CORES)),
        trace=trace,
    )
    hh = np.concatenate([r["h"] for r in res.results], axis=0)[:N_NODES]
    return np.ascontiguousarray(hh.astype(np.float32)), res.exec_time_ns


def kernel(**inputs) -> np.ndarray:
    hh, _ = run(inputs, trace=False)
    return hh

